# revision 21
# baseline (speedup 1.0000x reference)
"""Causal self-attention (B=2, T=2048, C=1024, H=16) on 8 TRN2 NeuronCores.

Sharding: tensor-parallel over heads. Each core owns 2 heads: it computes
q/k/v projections for its 128 feature columns, full causal attention for its
(batch, head) pairs, and a partial output projection against its 128 rows of
w_proj. The 8 partial [4096, 1024] outputs are summed on host and b_proj is
added once during that reduction.

v3: one software-pipelined instruction stream built to keep the PE
continuously busy (TRN2's PE p-state ramps 0.65->1.2->2.4 GHz and only
reaches full clock after ~3us of gap-free execution):

  * all matmul operands are bf16 (same 1 cycle/row as f32r for F>=256, but
    half the DMA/SBUF traffic, 2x DVE throughput, and no 4x penalty on
    small-F matmuls, which f32r has below F=256);
  * V is produced directly in [token, dim] layout (lhsT = xT tile) so no
    transpose pass exists; its bias is a K=1 ones-row matmul that opens the
    accumulation group; the whole 512-token V block is evacuated with one
    strided copy;
  * S / exp / AV are restricted to causally-valid columns on diagonal
    k-tiles; the triangle crossing block of both heads gets one fused
    multiply against a duplicated [128, 2x128] lower-triangle mask;
  * AV for k-tile kt issues two steps later (lag-2), so exp+mask latency
    never stalls the PE;
  * qkv-projection bundles, softmax normalization, and output-projection
    matmuls are deferred thunks on a global deadline queue, paced 1-2 per
    S/AV step, giving the PE independent work whenever ScalarE's exp lags;
  * the final q-chunk is processed as two 256-column halves so the first
    half's normalize+projection overlaps the second half's S/AV (shrinks
    the serial tail);
  * engine assignment: ScalarE = exp + half the y evacuations; DVE = the
    other PSUM evacuations + normalize; Pool(gpsimd) = causal-mask
    multiplies (SBUF-only: GPSIMD cannot touch PSUM on TRN2).

Softmax skips the max-subtraction: scores are ~N(0,1) (bounded ~+-6), far
inside fp32 exp range. Denominators fall out of the AV matmul via a ones
column appended to V per head; normalization happens on the tiny attn^T
tensor via a K=1 broadcast matmul + fast reciprocal.
"""

from collections import deque

import numpy as np
import ml_dtypes

import concourse.bass as bass
import concourse.mybir as mybir
import concourse.tile as tile
from concourse import bacc
from concourse.bass_utils import run_bass_kernel_spmd

F32 = mybir.dt.float32
BF16 = mybir.dt.bfloat16
EXP = mybir.ActivationFunctionType.Exp

B, T, C = 2, 2048, 1024
H, DH = 16, 64
NCORES = 8
FPC = (H // NCORES) * DH  # 128 q/k/v feature columns per core (2 heads)
N = B * T                 # 4096 tokens
NTT = N // 128            # 32 global 128-token tiles
NCT = C // 128            # 8 contraction tiles
SCALE = DH ** -0.5

# feature flags (HW bisect)
FUSE_V = True    # all 4 V tiles of a bundle in one PSUM tile + one 4D evac
ACT3D = True     # diagonal exp as one 3D-strided activation over both heads
MASK3D = True    # causal mask as one 3D-strided multiply over both heads
LAG2 = True      # AV issued two steps after its S (else one step)
SPLIT_LAST = False  # last q-chunk as two 256-col halves

_CACHE = {}


def _build():
    nc = bacc.Bacc(
        "TRN2",
        target_bir_lowering=False,
        debug=False,
        enable_asserts=True,
        num_devices=NCORES,
    )
    xT = nc.dram_tensor("xT", [C, N], BF16, kind="ExternalInput").ap()
    wq = nc.dram_tensor("wq", [C, FPC], BF16, kind="ExternalInput").ap()
    wk = nc.dram_tensor("wk", [C, FPC], BF16, kind="ExternalInput").ap()
    wv = nc.dram_tensor("wv", [C, FPC], BF16, kind="ExternalInput").ap()
    bq = nc.dram_tensor("bq", [FPC, 1], F32, kind="ExternalInput").ap()
    bk = nc.dram_tensor("bk", [FPC, 1], F32, kind="ExternalInput").ap()
    bv = nc.dram_tensor("bv", [1, FPC], BF16, kind="ExternalInput").ap()
    wp = nc.dram_tensor("wp", [FPC, C], BF16, kind="ExternalInput").ap()
    y = nc.dram_tensor("y", [N, C], BF16, kind="ExternalOutput").ap()

    with tile.TileContext(nc) as tc:
        with (
            tc.tile_pool(name="const", bufs=1) as cst,
            tc.tile_pool(name="qkvt", bufs=1) as qkvt,
            tc.tile_pool(name="xin", bufs=3) as xin,
            tc.tile_pool(name="ptile", bufs=5) as ptile,
            tc.tile_pool(name="attn", bufs=16) as attnp,
            tc.tile_pool(name="yout", bufs=4) as yout,
            tc.tile_pool(name="small", bufs=3) as small,
            tc.tile_pool(name="ps_s", bufs=2, space="PSUM") as ps_s,
            tc.tile_pool(name="ps_av", bufs=2, space="PSUM") as ps_av,
            tc.tile_pool(name="ps_misc", bufs=2, space="PSUM") as ps_misc,
        ):
            # ---- input DMAs; first x tile split in halves so the very
            # first qkv matmuls can start during the p-state ramp ----
            xT_view = xT.rearrange("(ct p) t -> p ct t", p=128)
            xt_tiles = {}

            def dma_xt(b, tj, split=False):
                xt = xin.tile([128, NCT, 512], BF16, tag="xt", name="xt")
                t0 = b * T + tj * 512
                if split:
                    nc.sync.dma_start(
                        out=xt[:, 0:4, :], in_=xT_view[:, 0:4, t0 : t0 + 512]
                    )
                    nc.sync.dma_start(
                        out=xt[:, 4:8, :], in_=xT_view[:, 4:8, t0 : t0 + 512]
                    )
                else:
                    nc.sync.dma_start(out=xt, in_=xT_view[:, :, t0 : t0 + 512])
                xt_tiles[(b, tj)] = xt

            # ---- weights / biases (wq first: needed by the first chain) ----
            w_sb = {}
            w_sb["q"] = cst.tile([128, NCT, FPC], BF16, tag="wq", name="wq")
            nc.sync.dma_start(
                out=w_sb["q"], in_=wq.rearrange("(ct p) f -> p ct f", p=128)
            )
            dma_xt(0, 0, split=True)
            for name, wap in (("k", wk), ("v", wv)):
                w_sb[name] = cst.tile(
                    [128, NCT, FPC], BF16, tag=f"w{name}", name=f"w{name}"
                )
                nc.sync.dma_start(
                    out=w_sb[name], in_=wap.rearrange("(ct p) f -> p ct f", p=128)
                )
            b_sb = {}
            for name, bap in (("q", bq), ("k", bk)):
                b_sb[name] = cst.tile([FPC, 1], F32, tag=f"b{name}", name=f"b{name}")
                nc.sync.dma_start(out=b_sb[name], in_=bap)
            bv_sb = cst.tile([1, FPC], BF16, tag="bv", name="bv")
            nc.sync.dma_start(out=bv_sb, in_=bv)
            wp_sb = cst.tile([FPC, C], BF16, tag="wp", name="wp")
            nc.sync.dma_start(out=wp_sb, in_=wp)

            # ---- constants ----
            onesf = cst.tile([128, 128], F32, tag="onesf", name="onesf")
            nc.vector.memset(onesf, 1.0)
            # lower-triangle mask (keep iff q-col >= k-partition), duplicated
            # side by side so one strided multiply covers both heads
            mtf = cst.tile([128, 128], F32, tag="mtf", name="mtf")
            nc.vector.memset(mtf, 1.0)
            nc.gpsimd.affine_select(
                out=mtf,
                in_=mtf,
                compare_op=mybir.AluOpType.is_ge,
                fill=0.0,
                base=0,
                pattern=[[1, 128]],
                channel_multiplier=-1,
            )
            M_tri2 = cst.tile([128, 2, 128], BF16, tag="mtri", name="mtri")
            nc.vector.tensor_copy(out=M_tri2[:, 0, :], in_=mtf)
            nc.vector.tensor_copy(out=M_tri2[:, 1, :], in_=mtf)
            # ones row at partition 64 (stationary of the K=1 denominator
            # broadcast matmul; partition 64 = where AV's ones-column lands)
            ones64 = cst.tile([128, 64], BF16, tag="ones64", name="ones64")
            nc.vector.tensor_copy(out=ones64[64:65, :], in_=onesf[64:65, 0:64])
            # ones row at partition 0 (stationary of the K=1 V-bias matmul)
            onesv = cst.tile([1, 128], BF16, tag="onesv", name="onesv")
            nc.vector.tensor_copy(out=onesv, in_=onesf[0:1, :])

            # ---- persistent activations ----
            QT = qkvt.tile([FPC, N], BF16, tag="QT", name="QT")
            KT = qkvt.tile([FPC, N], BF16, tag="KT", name="KT")
            # V with a ones column per head: per 128-token tile block of 130
            # cols: [64 V_h0 | 1 | 64 V_h1 | 1]
            V = qkvt.tile([128, NTT * 130], BF16, tag="V", name="V")
            V_blk = V.rearrange("p (kt two c) -> p kt two c", two=2, c=65)
            nc.vector.tensor_copy(out=V_blk[:, :, 0, 64], in_=onesf[:, 0:NTT])
            nc.vector.tensor_copy(out=V_blk[:, :, 1, 64], in_=onesf[:, 0:NTT])

            # ---- deferred-work queue: (deadline_entry, thunk) ----
            pending = deque()

            def drain(ci):
                rest = [item for item in pending if item[0] > ci]
                due = [item for item in pending if item[0] <= ci]
                pending.clear()
                pending.extend(rest)
                for _, th in due:
                    th()

            def pop_some(steps_left):
                if len(pending) > 2 * max(steps_left, 1):
                    n = 3
                elif len(pending) > steps_left:
                    n = 2
                else:
                    n = 1 if pending else 0
                for _ in range(min(n, len(pending))):
                    pending.popleft()[1]()

            # ---- qkv projection bundle -> thunks ----
            def qkv_thunks(b, tj):
                t0 = b * T + tj * 512

                def qk(name, out_sb):
                    def th():
                        xt = xt_tiles[(b, tj)]
                        acc = ps_misc.tile([128, 512], F32, tag="misc", name="acc")
                        for ct in range(NCT):
                            nc.tensor.matmul(
                                acc,
                                w_sb[name][:, ct, :],
                                xt[:, ct, :],
                                start=(ct == 0),
                                stop=(ct == NCT - 1),
                            )
                        nc.vector.tensor_scalar_add(
                            out_sb[:, t0 : t0 + 512], acc, b_sb[name]
                        )

                    return th

                def tv():
                    # all four 128-token V tiles of this 512-token bundle in
                    # one PSUM tile; one strided evacuation into V (the ones
                    # columns are skipped by the [tt, head, 64] view)
                    def th():
                        xt = xt_tiles[(b, tj)]
                        pv = ps_misc.tile([128, 512], F32, tag="misc", name="pv")
                        for tt in range(4):
                            sl = pv[:, tt * 128 : (tt + 1) * 128]
                            nc.tensor.matmul(
                                sl, onesv, bv_sb, start=True, stop=False,
                                skip_group_check=True,
                            )
                            for ct in range(NCT):
                                nc.tensor.matmul(
                                    sl,
                                    xt[:, ct, tt * 128 : (tt + 1) * 128],
                                    w_sb["v"][:, ct, :],
                                    start=False,
                                    stop=(ct == NCT - 1),
                                    skip_group_check=True,
                                )
                        gt = b * 16 + tj * 4
                        nc.vector.tensor_copy(
                            out=V_blk[:, gt : gt + 4, :, 0:64],
                            in_=pv.rearrange("p (tt two c) -> p tt two c",
                                             tt=4, two=2, c=64),
                        )

                    return th

                def tv_single(tt):
                    def th():
                        xt = xt_tiles[(b, tj)]
                        pv = ps_misc.tile([128, 128], F32, tag="misc", name="pv")
                        nc.tensor.matmul(pv, onesv, bv_sb, start=True, stop=False)
                        for ct in range(NCT):
                            nc.tensor.matmul(
                                pv,
                                xt[:, ct, tt * 128 : (tt + 1) * 128],
                                w_sb["v"][:, ct, :],
                                start=False,
                                stop=(ct == NCT - 1),
                            )
                        gt = b * 16 + tj * 4 + tt
                        nc.vector.tensor_copy(
                            out=V[:, 130 * gt : 130 * gt + 64], in_=pv[:, 0:64]
                        )
                        nc.vector.tensor_copy(
                            out=V[:, 130 * gt + 65 : 130 * gt + 129],
                            in_=pv[:, 64:128],
                        )

                    return th

                vthunks = [tv()] if FUSE_V else [tv_single(tt) for tt in range(4)]
                return [qk("q", QT), qk("k", KT)] + vthunks

            # ---- softmax-normalize + output projection (deferred) ----
            def bc_norm_thunk(avs, q_w, box):
                def th():
                    attn_t = attnp.tile([128, 512], BF16, tag="attn", name="attn")[:, 0:q_w]
                    attn1 = attnp.tile([64, 512], BF16, tag="attn1", name="attn1")[:, 0:q_w]
                    for h in range(2):
                        bcp = ps_misc.tile([64, 512], F32, tag="misc", name="bc")[:, 0:q_w]
                        nc.tensor.matmul(
                            bcp, ones64[64:65, :], avs[h][64:65, :],
                            start=True, stop=True,
                        )
                        rbc = small.tile([64, 512], F32, tag="rbc", name="rbc")[:, 0:q_w]
                        nc.vector.reciprocal_approx_fast(rbc, bcp)
                        tgt = attn_t[0:64, :] if h == 0 else attn1
                        nc.vector.tensor_mul(tgt, avs[h][0:64, :], rbc)
                    # head-1 half to partitions 64..127 (SBUF->SBUF DMA is the
                    # only cheap cross-partition path)
                    nc.scalar.dma_start(out=attn_t[64:128, :], in_=attn1)
                    box["attn"] = attn_t

                return th

            def proj_thunk(b, qc, q_lo, tt, cc, box, ybox):
                def th():
                    attn_t = box["attn"]
                    yp = ps_misc.tile([128, 512], F32, tag="misc", name="yp")
                    nc.tensor.matmul(
                        yp,
                        attn_t[:, tt * 128 : (tt + 1) * 128],
                        wp_sb[:, cc * 512 : (cc + 1) * 512],
                        start=True,
                        stop=True,
                    )
                    if cc == 0:
                        ybox["ysb"] = yout.tile([128, C], BF16, tag="ysb", name="ysb")
                    ysb = ybox["ysb"]
                    # evacuation alternates DVE / ScalarE so neither becomes
                    # the bottleneck
                    if cc == 0:
                        nc.vector.tensor_copy(
                            out=ysb[:, cc * 512 : (cc + 1) * 512], in_=yp
                        )
                    else:
                        nc.scalar.copy(ysb[:, cc * 512 : (cc + 1) * 512], yp)
                        t0 = b * T + qc * 512 + q_lo + tt * 128
                        nc.sync.dma_start(out=y[t0 : t0 + 128, :], in_=ysb)

                return th

            # ---- attention chunk (columns [q_lo, q_lo+q_w) of q-chunk qc):
            # S -> exp/mask -> AV (lag-2), deferred thunks sprinkled ----
            def emit_chunk(ci, b, qc, q_lo, q_w):
                drain(ci)
                q0 = b * T + qc * 512 + q_lo
                hi = qc * 512 + q_lo + q_w  # global end col within the batch
                nkt = hi // 128
                av = [
                    ps_av.tile([65, 512], F32, tag="av", name="av")[:, 0:q_w]
                    for _ in range(2)
                ]

                def emit_av(pt, lo, kt):
                    gkt = b * 16 + kt
                    for h in range(2):
                        nc.tensor.matmul(
                            av[h][:, lo:q_w] if lo else av[h],
                            V[:, 130 * gkt + 65 * h : 130 * gkt + 65 * h + 65],
                            pt[:, h * q_w + lo : (h + 1) * q_w],
                            start=(kt == 0),
                            stop=(kt == nkt - 1),
                            skip_group_check=True,
                        )

                backlog = deque()
                for kt in range(nkt):
                    # valid cols of this k-tile within [q_lo, q_lo+q_w)
                    lo = min(max(kt * 128 - (qc * 512 + q_lo), 0), q_w)
                    diag = kt * 128 >= qc * 512 + q_lo  # triangle block here
                    s = ps_s.tile([128, 1024], F32, tag="s", name="s")[:, 0 : 2 * q_w]
                    k0 = b * T + kt * 128
                    for h in range(2):
                        nc.tensor.matmul(
                            s[:, h * q_w + lo : (h + 1) * q_w],
                            KT[64 * h : 64 * h + 64, k0 : k0 + 128],
                            QT[64 * h : 64 * h + 64, q0 + lo : q0 + q_w],
                            start=True,
                            stop=True,
                        )
                    pt = ptile.tile([128, 1024], BF16, tag="pt", name="pt")[:, 0 : 2 * q_w]
                    if lo == 0:
                        nc.scalar.activation(out=pt, in_=s, func=EXP, scale=SCALE)
                    elif ACT3D:
                        sv = s.rearrange("p (two c) -> p two c", two=2)
                        pv_ = pt.rearrange("p (two c) -> p two c", two=2)
                        nc.scalar.activation(
                            out=pv_[:, :, lo:q_w],
                            in_=sv[:, :, lo:q_w],
                            func=EXP,
                            scale=SCALE,
                        )
                    else:
                        for h in range(2):
                            nc.scalar.activation(
                                out=pt[:, h * q_w + lo : (h + 1) * q_w],
                                in_=s[:, h * q_w + lo : (h + 1) * q_w],
                                func=EXP,
                                scale=SCALE,
                            )
                    if diag:  # triangle crossing block, both heads
                        if MASK3D:
                            ptv = pt.rearrange("p (two c) -> p two c", two=2)
                            nc.gpsimd.tensor_mul(
                                ptv[:, :, lo : lo + 128],
                                ptv[:, :, lo : lo + 128],
                                M_tri2,
                            )
                        else:
                            for h in range(2):
                                sl = slice(h * q_w + lo, h * q_w + lo + 128)
                                nc.gpsimd.tensor_mul(
                                    pt[:, sl], pt[:, sl], M_tri2[:, 0, :]
                                )
                    pop_some(nkt - 1 - kt)
                    backlog.append((pt, lo, kt))
                    if len(backlog) > (2 if LAG2 else 1):
                        emit_av(*backlog.popleft())
                while backlog:
                    emit_av(*backlog.popleft())
                # evacuate AV PSUM (numerators + denominators) to SBUF bf16
                avs = []
                for h in range(2):
                    a = attnp.tile([65, 512], BF16, tag="avs", name="avs")[:, 0:q_w]
                    nc.vector.tensor_copy(a, av[h])
                    avs.append(a)
                return avs

            # ---- main pipeline ----
            for th in qkv_thunks(0, 0):
                th()
            dma_xt(0, 1)

            # last q-chunk split into two column halves so its normalize +
            # projection overlaps the second half's S/AV
            entries = [(b, qc, 0, 512) for b in range(B) for qc in range(4)]
            if SPLIT_LAST:
                entries[-1] = (1, 3, 0, 256)
                entries.append((1, 3, 256, 256))
            NB = 8  # qkv bundles, one per 512-token tile

            for ci, (b, qc, q_lo, q_w) in enumerate(entries):
                if ci + 2 < NB:  # x prefetch for bundle ci+2
                    bn, tjn = divmod(ci + 2, 4)
                    pending.append((ci, lambda bn=bn, tjn=tjn: dma_xt(bn, tjn)))
                if ci + 1 < NB:  # qkv bundle ci+1, due before entry ci+1
                    bn, tjn = divmod(ci + 1, 4)
                    for th in qkv_thunks(bn, tjn):
                        pending.append((ci + 1, th))
                avs = emit_chunk(ci, b, qc, q_lo, q_w)
                box = {}
                pending.append((ci + 3, bc_norm_thunk(avs, q_w, box)))
                for tt in range(q_w // 128):
                    ybox = {}
                    for cc in range(2):
                        pending.append(
                            (ci + 3, proj_thunk(b, qc, q_lo, tt, cc, box, ybox))
                        )
            drain(len(entries) + 3)
            assert not pending

    nc.compile()
    return nc


def _get_nc():
    if "nc" not in _CACHE:
        _CACHE["nc"] = _build()
    return _CACHE["nc"]


def _bf16(x: np.ndarray) -> np.ndarray:
    return np.ascontiguousarray(x).astype(ml_dtypes.bfloat16)


def _run(inputs, **spmd_kwargs):
    x = np.asarray(inputs["x"], dtype=np.float32)
    w_qkv = np.asarray(inputs["w_qkv"], dtype=np.float32)
    b_qkv = np.asarray(inputs["b_qkv"], dtype=np.float32)
    w_proj = np.asarray(inputs["w_proj"], dtype=np.float32)
    b_proj = np.asarray(inputs["b_proj"], dtype=np.float32)

    nc = _get_nc()

    xT = _bf16(x.reshape(N, C).T)
    in_maps = []
    for i in range(NCORES):
        f0 = i * FPC
        in_maps.append(
            {
                "xT": xT,
                "wq": _bf16(w_qkv[:, f0 : f0 + FPC]),
                "wk": _bf16(w_qkv[:, C + f0 : C + f0 + FPC]),
                "wv": _bf16(w_qkv[:, 2 * C + f0 : 2 * C + f0 + FPC]),
                "bq": np.ascontiguousarray(
                    b_qkv[f0 : f0 + FPC], dtype=np.float32
                ).reshape(FPC, 1),
                "bk": np.ascontiguousarray(
                    b_qkv[C + f0 : C + f0 + FPC], dtype=np.float32
                ).reshape(FPC, 1),
                "bv": _bf16(b_qkv[2 * C + f0 : 2 * C + f0 + FPC]).reshape(1, FPC),
                "wp": _bf16(w_proj[f0 : f0 + FPC, :]),
            }
        )

    res = run_bass_kernel_spmd(nc, in_maps, core_ids=list(range(NCORES)), **spmd_kwargs)
    acc = np.zeros((N, C), dtype=np.float64)
    for om in res.results:
        acc += np.asarray(om["y"]).astype(np.float64)
    out = (acc + b_proj.astype(np.float64)).astype(np.float32)
    return out.reshape(B, T, C), res


def kernel(**inputs) -> np.ndarray:
    out, _ = _run(inputs)
    return out
